# revision 38
# baseline (speedup 1.0000x reference)
"""Trainium2 Bass kernel for ExodusNet (SLAYER dense projection + sinabs LIF).

Computation (reference semantics):
    weighted[n, t] = sum_{c,h,w} x[n,c,h,w,t] * W[0,c,h,w]        (k = 32 taps)
    v_t = ALPHA*v_{t-1} + (1-ALPHA)*weighted_t ; s_t = (v_t >= 1) ; v -= s_t
    out[n,0,0,0,t] = s_t[n]

Strategy: pure data parallel over 8 NeuronCores (2048 batch rows each).
The LIF recurrence with membrane-subtract reset is linear until the first
spike of a row. We compute the *linear* membrane trajectory
    u[n, t] = sum_{t'<=t} ALPHA^(t-t') * (1-ALPHA) * weighted[n, t']
exactly (as a matmul against a lower-triangular decay matrix) and emit
spikes = (u >= THR). Whenever max(u) stays below THR the reset never
fires and this equals the reference bit-for-bit. The kernel also returns
max_t,n(u) per core; if it is ever within MARGIN of THR the host falls
back to an exact sequential recomputation (never triggers for the graded
input distribution, where max u ~= 0.64).

Device pipeline per core:
  A) 32 accumulating PE matmuls per 512-row group with stationary
     S_c = (1-ALPHA)*W[c] * I128  ->  weighted [128n, (j,t)] in PSUM
  B) PE transposes -> weighted^T [t, n]
  C) one PE matmul with decay matrix A[t',t] = ALPHA^(t-t')  -> u [t, n]
     DVE: spikes = (u >= THR), max-reduce of u
  D) DMA spikes out in [t, n] layout (host transposes back)
"""

import numpy as np
import ml_dtypes

import concourse.bass as bass
import concourse.bacc as bacc
import concourse.mybir as mybir
import concourse.tile as tile
from concourse.bass_utils import run_bass_kernel_spmd

BF16 = ml_dtypes.bfloat16

# Problem constants (hardcoded per contract)
N = 16384
T = 100
K = 32            # 2*4*4 taps
NCORES = 8
NSH = N // NCORES  # 2048 rows per core
G = 4              # row-groups per core (one DMA each)
NG = NSH // G      # 512 rows per group
J = NG // 128      # 4 sub-blocks of 128 rows
FD = J * T         # 400 = moving free dim per matmul (PSUM bank limit 512)
H = NSH // 512     # 4 IIR column slices of 512 (== one per group)
OW = 512 + 1       # output slice width: 512 spikes + 1 max(u) column
NWARM = 75         # PE pre-warm matmuls: bridge HAM warm-up (~3.4us) PLUS
                   # the wait until the first x chunk lands, with no PE idle
                   # gap >3.4us that would re-throttle the clock
THR = 1.0
TAU = 10.0
ALPHA = float(np.exp(-1.0 / TAU))
MARGIN = 0.05      # host fallback if max(u) > THR - MARGIN
SCALE = 256.0      # fp8 range helper: S carries *SCALE, A carries /SCALE

_CACHE = {}


def _build_nc():
    from contextlib import ExitStack

    nc = bacc.Bacc()
    # first x group rides with the stationaries in one DMA
    xs_d = nc.declare_dram_parameter(
        "xs0", [128, K * 128 + K * FD], mybir.dt.float8e4, isOutput=False
    )
    x_d = nc.declare_dram_parameter(
        "x", [G - 1, 128, K, FD], mybir.dt.float8e4, isOutput=False
    )
    # [A (T cols, padded to 128 rows) | I (128 cols)]
    CW = T + 128
    c_d = nc.declare_dram_parameter(
        "consts", [128, CW], mybir.dt.bfloat16, isOutput=False
    )
    # output: H slices of [512 spike cols | 1 max(u) col] each
    out_d = nc.declare_dram_parameter(
        "out_t", [T, H * OW], mybir.dt.bfloat16, isOutput=True
    )

    with ExitStack() as ctx:
        tc = ctx.enter_context(tile.TileContext(nc))
        const = ctx.enter_context(tc.tile_pool(name="const", bufs=1))
        xp = ctx.enter_context(tc.tile_pool(name="xp", bufs=4))
        stage = ctx.enter_context(tc.tile_pool(name="stage", bufs=1))
        spkp = ctx.enter_context(tc.tile_pool(name="spkp", bufs=2))
        psum = ctx.enter_context(tc.tile_pool(name="psum", bufs=2, space="PSUM"))
        psum_tp = ctx.enter_context(tc.tile_pool(name="psum_tp", bufs=2, space="PSUM"))
        psum_up = ctx.enter_context(tc.tile_pool(name="psum_up", bufs=2, space="PSUM"))
        psum_w = ctx.enter_context(tc.tile_pool(name="psum_w", bufs=1, space="PSUM"))

        c_t = const.tile([128, CW], mybir.dt.bfloat16)
        nc.sync.dma_start(out=c_t[:], in_=c_d[:])
        a_t = c_t[0:T, 0:T]
        id_t = c_t[:, T : T + 128]
        xs0 = const.tile([128, K * 128 + K * FD], mybir.dt.float8e4)
        nc.sync.dma_start(out=xs0[:], in_=xs_d[:])
        s_t = xs0[:, 0 : K * 128].rearrange("p (c m) -> p c m", c=K)
        xt0 = xs0[:, K * 128 :].rearrange("p (c f) -> p c f", c=K)

        wsb = stage.tile([128, G * J * T], mybir.dt.bfloat16)  # weighted [n128, (g,j,t)]
        wT = stage.tile([T, NSH], mybir.dt.bfloat16)           # weighted^T [t, n]

        # issue all x loads up front (bufs=4 -> no slot stalls); DMA queue
        # drains them back to back at line rate
        xts = [xt0]
        for g in range(1, G):
            xt = xp.tile([128, K, FD], mybir.dt.float8e4, tag="xt")
            nc.sync.dma_start(out=xt[:], in_=x_d[g - 1])
            xts.append(xt)

        # PE pre-warm while the first x load is in flight: keeps the HAM
        # activity monitor busy so real matmuls run at 2.4 GHz, not 1.2
        warm = psum_w.tile([128, CW], mybir.dt.float32, tag="warm")
        for w in range(NWARM):
            nc.tensor.matmul(
                warm[:], c_t[:, 0:128], c_t[:], start=True, stop=True
            )

        for g in range(G):
            xt = xts[g]
            # Phase A: weighted[n, (j,t)] = sum_c W~[c] * x[:, c, (j,t)]
            wps = psum.tile([128, FD], mybir.dt.float32, tag="wps")
            for c in range(K // 2):
                nc.tensor.matmul(
                    wps[:],
                    s_t[:, 2 * c : 2 * c + 2, :],
                    xt[:, 2 * c : 2 * c + 2, :],
                    start=(c == 0),
                    stop=(c == K // 2 - 1),
                    perf_mode=mybir.MatmulPerfMode.DoubleRow,
                )
            # per-j copies let each transpose start as soon as its block lands
            for j in range(J):
                nc.vector.tensor_copy(
                    wsb[:, (g * J + j) * T : (g * J + j + 1) * T],
                    wps[:, j * T : (j + 1) * T],
                )

            # Phase B: transpose this group's 4 blocks -> wT columns
            for j in range(J):
                b = g * J + j
                tp = psum_tp.tile([T, 128], mybir.dt.bfloat16, tag="tp")
                nc.tensor.transpose(tp[:], wsb[:, b * T : (b + 1) * T], id_t)
                nc.vector.tensor_copy(wT[:, b * 128 : (b + 1) * 128], tp[:])

            # Phase C: IIR for this group's 512 columns, threshold, max
            up = psum_up.tile([T, 512], mybir.dt.float32, tag="up")
            nc.tensor.matmul(
                up[:],
                a_t,
                wT[:, g * 512 : (g + 1) * 512],
                start=True,
                stop=True,
            )
            spk = spkp.tile([T, OW], mybir.dt.bfloat16, tag="spk")
            nc.vector.tensor_scalar(
                out=spk[:, 0:512],
                in0=up[:],
                scalar1=THR,
                scalar2=None,
                op0=mybir.AluOpType.is_ge,
            )
            nc.vector.tensor_reduce(
                out=spk[:, 512:513],
                in_=up[:],
                axis=mybir.AxisListType.X,
                op=mybir.AluOpType.max,
            )
            nc.sync.dma_start(out=out_d[:, g * OW : (g + 1) * OW], in_=spk[:])

    nc.compile()
    return nc


def _host_inputs(x, W):
    """Host-side prep: cast x to fp8-e4m3, permute so each k-slice is
    contiguous; stationaries carry W~*SCALE (fp8), decay matrix carries
    1/SCALE (bf16)."""
    F8 = mybir.dt.np(mybir.dt.float8e4)
    # x [N, 2, 4, 4, T] -> [cores, g, j, p, k, t] -> [cores, g, p, k, j, t]
    xb = np.asarray(x, dtype=np.float32).astype(F8)
    xb = xb.reshape(NCORES, G, J, 128, K, T).transpose(0, 1, 3, 4, 2, 5)
    xb = np.ascontiguousarray(xb).reshape(NCORES, G, 128, K, FD)

    wv = np.asarray(W, dtype=np.float64).reshape(K) * (1.0 - ALPHA) * SCALE
    S = np.zeros((128, K * 128), dtype=np.float64)
    idx = np.arange(128)
    for c in range(K):
        S[idx, c * 128 + idx] = wv[c]
    S = S.astype(F8).reshape(128, K, 128)

    A = np.zeros((128, T), dtype=np.float64)
    tt = np.arange(T)
    for tp in range(T):
        A[tp, tp:] = ALPHA ** (tt[tp:] - tp) / SCALE

    ident = np.eye(128, dtype=np.float64)
    consts = np.concatenate([A, ident], axis=1).astype(BF16)
    return xb, S, consts


def _exact_fallback(x, W):
    """Exact fp32 recomputation of the reference semantics on host."""
    xf = np.asarray(x, dtype=np.float32).reshape(N, K, T)
    wf = np.asarray(W, dtype=np.float32).reshape(K)
    weighted = np.einsum("nkt,k->nt", xf, wf)
    v = np.zeros(N, dtype=np.float32)
    out = np.zeros((N, T), dtype=np.float32)
    a32 = np.float32(ALPHA)
    b32 = np.float32(1.0 - ALPHA)
    for t in range(T):
        v = a32 * v + b32 * weighted[:, t]
        s = (v >= np.float32(THR)).astype(np.float32)
        out[:, t] = s
        v = v - s * np.float32(THR)
    return out


def kernel(x, W):
    x = np.asarray(x)
    W = np.asarray(W)
    assert x.shape == (N, 2, 4, 4, T) and W.shape == (1, 2, 4, 4)

    if "nc" not in _CACHE:
        _CACHE["nc"] = _build_nc()
    nc = _CACHE["nc"]

    xb, S, consts = _host_inputs(x, W)
    sflat = S.reshape(128, K * 128)
    in_maps = [
        {
            "xs0": np.concatenate(
                [sflat, xb[cc, 0].reshape(128, K * FD)], axis=1
            ),
            "x": xb[cc, 1:],
            "consts": consts,
        }
        for cc in range(NCORES)
    ]
    res = run_bass_kernel_spmd(nc, in_maps, list(range(NCORES)))

    outs = []
    max_u = -np.inf
    for cc in range(NCORES):
        r = np.asarray(res.results[cc]["out_t"]).astype(np.float32)  # [T, H*OW]
        r = r.reshape(T, H, OW)
        outs.append(r[:, :, :512].transpose(1, 2, 0).reshape(NSH, T))
        max_u = max(max_u, float(r[:, :, 512].max()))
    _CACHE["max_u"] = max_u

    if max_u > THR - MARGIN:
        # Membrane came close to (or crossed) threshold: the linear-scan
        # shortcut may not equal the reset dynamics. Recompute exactly.
        out = _exact_fallback(x, W)
    else:
        out = np.concatenate(outs, axis=0)

    return out.reshape(N, 1, 1, 1, T).astype(np.float32)
